# revision 42
# baseline (speedup 1.0000x reference)
"""Sigmoid-attention block on 8 TRN2 NeuronCores, v3.

Sharding: core c = (batch b=c//2, head-half hh=c%2).  Each core computes
Q^T/K^T directly in transposed layout (W^T @ x^T, no PE transposes), ropes
them in-place (aligned mul + 32-partition shift copies), and runs causal
sigmoid attention for its 6 heads with query-window-restricted diagonal
chunks.

Epilogue v3: token-split via AllToAll.  Within a pair, rank j owns the
j-th 256-token half of every 512-query block.  Each core contributes
shard j = its own [attn_out | silu(U)] restricted to rank-j's tokens;
the A2A hands back both ranks' heads for the core's OWN tokens at static
shard indices (agout[r] = heads of rank r), so the program stays fully
SPMD-symmetric.  LN stats, LayerNorm, gating and the FULL 768-column
output projection then run locally on 256 tokens per block -- half the
vector work and half the collective wire bytes of the v2 all-gather
scheme, with no second collective.

Emission interleaves projection Mtile units INTO the attention pair loop
(the attention chunk pipeline is ScalarE-paced, interleaved projection
matmuls soak up the PE slack), epilogue loads (B1) are prefetched ahead
of the epilogue compute (B2), qb3's A2A is chunked per head-pair so each
chunk flies while the next pair's attention runs, and ~20 warm-up
matmuls on a zeroed tile run during the initial DMA so the PE HAM
clock-gate is released (2.4 GHz) before the first real projection.
Inputs are host-packed so every big load is one contiguous-per-partition
DMA descriptor chain (>=6KB/partition).
"""

import numpy as np
import ml_dtypes

import concourse.bass as bass
import concourse.bacc as bacc
import concourse.mybir as mybir
import concourse.tile as tile
from concourse import bass_utils

BF16 = mybir.dt.bfloat16
F32 = mybir.dt.float32
AF = mybir.ActivationFunctionType

S = 2048          # sequence length
HID = 768         # hidden
NH = 6            # heads per core
NPAIR = 3         # head pairs per core
D = 64            # head dim
RB = 512          # row block (query block size)
HB = 256          # per-rank token half of a block
NQB = 4           # query blocks
LN_EPS = 1e-8
N_CORES = 8


def _rope_tables():
    inv_freq = 1.0 / (10000.0 ** (np.arange(0, D, 2, dtype=np.float64) / D))
    t = np.arange(S, dtype=np.float64)
    freqs = np.outer(t, inv_freq)                      # [S, 32]
    emb = np.concatenate([freqs, freqs], axis=-1)      # [S, 64]
    return np.cos(emb).astype(np.float32), np.sin(emb).astype(np.float32)


def build_nc(ndev, pairs):
    nc = bacc.Bacc("TRN2", target_bir_lowering=False, debug=False,
                   num_devices=ndev)

    def din(name, shape, dt):
        return nc.dram_tensor(name, shape, dt, kind="ExternalInput").ap()

    xp = din("xp", [128, 4, 6, RB], BF16)          # x^T packed per block
    w_qku = din("w_qku", [128, 6, 1152], BF16)     # Q(384) | K(384) | U(384)
    w_u2 = din("w_u2", [128, 6, HID], BF16)        # U all 768 cols (global)
    wv = din("wv", [128, 6, 384], BF16)
    w_out = din("w_out", [128, 6, 384], BF16)      # gamma-folded, own cols
    cosT2 = din("cosT2", [128, S], BF16)           # cos^T stacked 2x
    sinfT2 = din("sinfT2", [128, S], BF16)         # sign-folded sin^T 2x
    maskf = din("maskf", [128, 128], BF16)         # (col >= row), diag 128
    ones_k = din("ones_k", [128, 1], BF16)
    residT = din("residT", [128, 3, S], BF16)      # x^T half + b_out, packed
    out = nc.dram_tensor("out", [128, 4, 3, RB], F32,
                         kind="ExternalOutput").ap()

    with tile.TileContext(nc) as tc:
        _emit(nc, tc, pairs, xp, w_qku, w_u2, wv, w_out, cosT2, sinfT2,
              maskf, ones_k, residT, out)
    nc.compile()
    return nc


def _emit(nc, tc, pairs, xp, w_qku, w_u2, wv, w_out, cosT2, sinfT2,
          maskf, ones_k, residT, out):
    from contextlib import ExitStack
    es = ExitStack()
    with es:
        # ---- resident SBUF tensors -----------------------------------
        res = es.enter_context(tc.tile_pool(name="resident", bufs=1))
        wqku_sb = res.tile([128, 6, 1152], BF16, tag="wqku")
        wu2_sb = res.tile([128, 6, HID], BF16, tag="wu2")
        wv_sb = res.tile([128, 6, 384], BF16, tag="wv")
        wout_sb = res.tile([128, 6, 384], BF16, tag="wout")
        cos_sb = res.tile([128, S], BF16, tag="cos")
        sinf_sb = res.tile([128, S], BF16, tag="sinf")
        maskf_sb = res.tile([128, 128], BF16, tag="maskf")
        ones_k_sb = res.tile([128, 1], BF16, tag="onesk")
        ones_r_sb = res.tile([1, 128], BF16, tag="onesr")
        eps_t = res.tile([1, 1], F32, tag="eps")
        warm_sb = res.tile([128, RB], BF16, tag="warm")
        qt_sb = res.tile([128, NPAIR, S], BF16, tag="qt")   # roped Q^T
        kt_sb = res.tile([128, NPAIR, S], BF16, tag="kt")   # roped K^T
        v_sb = res.tile([128, 16, 384], BF16, tag="v")      # V row layout
        ut_sb = res.tile([128, 3, 2 * RB], BF16, tag="ut")  # silu(U), blk 0-1
        ut2_sb = res.tile([128, 6, 2 * RB], BF16, tag="ut2")  # blk 2-3, full
        ao_sb = res.tile([128, 3, S], BF16, tag="ao")       # attn out^T half

        # ---- pools ---------------------------------------------------
        scp = es.enter_context(tc.tile_pool(name="scp", bufs=2,
                                            space="PSUM"))   # 4 banks
        avp = es.enter_context(tc.tile_pool(name="avp", bufs=1,
                                            space="PSUM"))   # 1 bank
        atp = es.enter_context(tc.tile_pool(name="atp", bufs=7))
        sb1 = es.enter_context(tc.tile_pool(name="p1sb", bufs=2))
        dram = es.enter_context(tc.tile_pool(name="agdram", bufs=2,
                                             space="DRAM"))
        pp = es.enter_context(tc.tile_pool(name="p1psum", bufs=2,
                                           space="PSUM"))

        # ---- HAM warm-up: PE busy during initial DMA -----------------
        nc.gpsimd.memset(warm_sb[:], 0.0)
        nc.gpsimd.memset(ones_r_sb[:], 1.0)
        nc.gpsimd.memset(eps_t[:], LN_EPS)
        wu = pp.tile([128, RB], F32, tag="pp", name="wu")
        for i in range(20):
            nc.tensor.matmul(wu[:], warm_sb[:, 0:128], warm_sb[:],
                             start=(i == 0), stop=(i == 19))

        # xT lives in a 2-deep rotating block pool: block nb is only
        # needed during proj_block(nb)
        xT_blk = [sb1.tile([128, 6, RB], BF16, tag="xT", bufs=2,
                           name=f"xT{nb}") for nb in range(4)]

        # load order: first seq-block + QKU weights first so the first
        # matmuls start early; every big tensor is one packed DMA
        nc.sync.dma_start(out=xT_blk[0][:], in_=xp[:, 0])
        nc.scalar.dma_start(out=wqku_sb[:], in_=w_qku[:])
        nc.scalar.dma_start(out=cos_sb[:], in_=cosT2[:])
        nc.scalar.dma_start(out=sinf_sb[:], in_=sinfT2[:])
        nc.scalar.dma_start(out=wv_sb[:], in_=wv[:])
        nc.scalar.dma_start(out=maskf_sb[:], in_=maskf[:])
        nc.scalar.dma_start(out=ones_k_sb[:], in_=ones_k[:])
        for nb in range(1, 4):
            nc.sync.dma_start(out=xT_blk[nb][:], in_=xp[:, nb])
        nc.scalar.dma_start(out=wu2_sb[:], in_=w_u2[:])
        nc.scalar.dma_start(out=wout_sb[:], in_=w_out[:])

        # ------------- phase 1 helpers --------------------------------
        def proj_rope(pp, role, p, nb):
            """role 0=Q, 1=K: project pair p's 128 ^T-rows for seq block
            nb and rope into qt/kt."""
            sl = slice(nb * RB, (nb + 1) * RB)
            pq = pp.tile([128, RB], F32, tag="pp")
            c0 = role * 384 + p * 128
            for k in range(6):
                nc.tensor.matmul(pq[:], wqku_sb[:, k, c0:c0 + 128],
                                 xT_blk[nb][:, k, :], start=(k == 0),
                                 stop=(k == 5))
            # rope: w = pq*g (aligned), shift w across 32-blocks (single-
            # input copies -- the only partition-base-mismatch the HW
            # verifier allows), out = pq*cos + shifted(w)
            w = sb1.tile([128, RB], BF16, tag="w")
            ws = sb1.tile([128, RB], BF16, tag="ws")
            t2 = sb1.tile([128, RB], BF16, tag="t2")
            nc.vector.tensor_mul(w[:], pq[:], sinf_sb[:, sl])
            nc.vector.tensor_copy(ws[0:32, :], w[32:64, :])
            nc.vector.tensor_copy(ws[32:64, :], w[0:32, :])
            nc.vector.tensor_copy(ws[64:96, :], w[96:128, :])
            nc.vector.tensor_copy(ws[96:128, :], w[64:96, :])
            nc.vector.tensor_mul(t2[:], pq[:], cos_sb[:, sl])
            dst = kt_sb if role else qt_sb
            nc.vector.tensor_add(dst[:, p, sl], t2[:], ws[:])

        def proj_u(pp, ct, nb):
            """own-half silu(U) for blocks 0-1 (gathered later)."""
            sl = slice(nb * RB, (nb + 1) * RB)
            pu = pp.tile([128, RB], F32, tag="pp", name="pu")
            c0 = 768 + ct * 128
            for k in range(6):
                nc.tensor.matmul(pu[:], wqku_sb[:, k, c0:c0 + 128],
                                 xT_blk[nb][:, k, :], start=(k == 0),
                                 stop=(k == 5))
            usig = sb1.tile([128, RB], BF16, tag="usig")
            nc.scalar.activation(usig[:], pu[:], AF.Sigmoid)
            nc.vector.tensor_mul(ut_sb[:, ct, sl], usig[:], pu[:])

        def proj_u2(pp, ct, nb):
            """full-hidden silu(U) (global ct order) for blocks 2-3 --
            both cores compute both halves so the late gathers carry
            only attn_out."""
            pu = pp.tile([128, RB], F32, tag="pp", name="pu2")
            for k in range(6):
                nc.tensor.matmul(pu[:], wu2_sb[:, k, ct * 128:(ct + 1) * 128],
                                 xT_blk[nb][:, k, :], start=(k == 0),
                                 stop=(k == 5))
            usig = sb1.tile([128, RB], BF16, tag="usig", name="usig2")
            nc.scalar.activation(usig[:], pu[:], AF.Sigmoid)
            nc.vector.tensor_mul(ut2_sb[:, ct, (nb - 2) * RB:(nb - 1) * RB],
                                 usig[:], pu[:])

        def proj_v(pp, rt):
            pv = pp.tile([128, RB], F32, tag="pp", name="pv")
            r4 = rt % 4
            for k in range(6):
                nc.tensor.matmul(pv[:, 0:384],
                                 xT_blk[rt // 4][:, k, r4 * 128:(r4 + 1) * 128],
                                 wv_sb[:, k, :], start=(k == 0), stop=(k == 5))
            nc.vector.tensor_copy(v_sb[:, rt, :], pv[:, 0:384])

        def proj_block(pp, nb):
            for p in range(NPAIR):
                proj_rope(pp, 1, p, nb)      # K first
            for p in range(NPAIR):
                proj_rope(pp, 0, p, nb)
            for rt in range(4 * nb, 4 * nb + 4):
                proj_v(pp, rt)
            for ct in range(3):
                proj_u(pp, ct, nb)

        def proj_units(pp, nb):
            """projection of seq-block nb as schedulable units."""
            for p in range(NPAIR):
                yield lambda p=p: proj_rope(pp, 1, p, nb)
            for p in range(NPAIR):
                yield lambda p=p: proj_rope(pp, 0, p, nb)
            for rt in range(4 * nb, 4 * nb + 4):
                yield lambda rt=rt: proj_v(pp, rt)
            if nb < 2:
                for ct in range(3):
                    yield lambda ct=ct: proj_u(pp, ct, nb)
            else:
                for ct in range(6):
                    yield lambda ct=ct: proj_u2(pp, ct, nb)

        # ------------- attention --------------------------------------
        def attn_pair(qb, p):
            q0 = qb * RB
            av = avp.tile([128, RB], F32, tag="av")

            def av_nondiag(kc, at):
                for h in range(2):
                    b0 = 64 * h
                    nc.tensor.matmul(
                        av[b0:b0 + 64, :],
                        v_sb[:, kc, (2 * p + h) * 64:(2 * p + h + 1) * 64],
                        at[:, h * RB:(h + 1) * RB],
                        start=(kc == 0), stop=False, skip_group_check=True)

            prev = None                       # (kc, at) pending A@V
            for kc in range(4 * qb):          # fully unmasked chunks
                sc = scp.tile([128, 1024], F32, tag="sc")
                at = atp.tile([128, 1024], BF16, tag="at")
                for h in range(2):
                    b0 = 64 * h
                    nc.tensor.matmul(
                        sc[:, h * RB:(h + 1) * RB],
                        kt_sb[b0:b0 + 64, p, kc * 128:(kc + 1) * 128],
                        qt_sb[b0:b0 + 64, p, q0:q0 + RB],
                        start=True, stop=True)
                nc.scalar.activation(at[:], sc[:], AF.Sigmoid, scale=0.125)
                if prev is not None:
                    av_nondiag(*prev)
                prev = (kc, at)
            # diagonal chunks t=0..3: query windows 512/384/256/128
            kcd = 4 * qb
            # D0: t=0, full window, one [128,1024] tile like nondiag
            sc = scp.tile([128, 1024], F32, tag="sc", name="scd0")
            at0 = atp.tile([128, 1024], BF16, tag="at", name="atd0")
            for h in range(2):
                b0 = 64 * h
                nc.tensor.matmul(
                    sc[:, h * RB:(h + 1) * RB],
                    kt_sb[b0:b0 + 64, p, kcd * 128:(kcd + 1) * 128],
                    qt_sb[b0:b0 + 64, p, q0:q0 + RB],
                    start=True, stop=True)
            nc.scalar.activation(at0[:], sc[:], AF.Sigmoid, scale=0.125)
            for h in range(2):
                # only the first 128 query cols of the window are masked
                nc.vector.tensor_mul(at0[:, h * RB:h * RB + 128],
                                     at0[:, h * RB:h * RB + 128], maskf_sb[:])
            if prev is not None:
                av_nondiag(*prev)
                prev = None
            # D1: t=1, window [128,512): per-head 384 cols at h*512
            sc1 = scp.tile([128, 1024], F32, tag="sc", name="scd1")
            at1 = atp.tile([128, 1024], BF16, tag="at", name="atd1")
            for h in range(2):
                b0 = 64 * h
                nc.tensor.matmul(
                    sc1[:, h * RB:h * RB + 384],
                    kt_sb[b0:b0 + 64, p, (kcd + 1) * 128:(kcd + 2) * 128],
                    qt_sb[b0:b0 + 64, p, q0 + 128:q0 + RB],
                    start=True, stop=True)
                nc.scalar.activation(at1[:, h * RB:h * RB + 384],
                                     sc1[:, h * RB:h * RB + 384],
                                     AF.Sigmoid, scale=0.125)
                nc.vector.tensor_mul(at1[:, h * RB:h * RB + 128],
                                     at1[:, h * RB:h * RB + 128],
                                     maskf_sb[:])
            # D2: t=2 (N=256) + t=3 (N=128): per-head 384 cols at h*512
            sc2 = scp.tile([128, 1024], F32, tag="sc", name="scd2")
            at2 = atp.tile([128, 1024], BF16, tag="at", name="atd2")
            for h in range(2):
                b0 = 64 * h
                nc.tensor.matmul(
                    sc2[:, h * RB:h * RB + 256],
                    kt_sb[b0:b0 + 64, p, (kcd + 2) * 128:(kcd + 3) * 128],
                    qt_sb[b0:b0 + 64, p, q0 + 256:q0 + RB],
                    start=True, stop=True)
                nc.tensor.matmul(
                    sc2[:, h * RB + 256:h * RB + 384],
                    kt_sb[b0:b0 + 64, p, (kcd + 3) * 128:(kcd + 4) * 128],
                    qt_sb[b0:b0 + 64, p, q0 + 384:q0 + RB],
                    start=True, stop=True)
                nc.scalar.activation(at2[:, h * RB:h * RB + 384],
                                     sc2[:, h * RB:h * RB + 384],
                                     AF.Sigmoid, scale=0.125)
                # masked cols: [0:128] of the 256-wide t=2 window, and
                # all 128 of the t=3 window at offset 256
                nc.vector.tensor_mul(at2[:, h * RB:h * RB + 128],
                                     at2[:, h * RB:h * RB + 128],
                                     maskf_sb[:])
                nc.vector.tensor_mul(at2[:, h * RB + 256:h * RB + 384],
                                     at2[:, h * RB + 256:h * RB + 384],
                                     maskf_sb[:])
            # ---- diagonal A @ V --------------------------------------
            for h in range(2):
                b0 = 64 * h
                vh = lambda kc: v_sb[:, kc, (2 * p + h) * 64:(2 * p + h + 1) * 64]
                nc.tensor.matmul(av[b0:b0 + 64, :], vh(kcd),
                                 at0[:, h * RB:(h + 1) * RB],
                                 start=(qb == 0), stop=False,
                                 skip_group_check=True)
                nc.tensor.matmul(av[b0:b0 + 64, 128:RB], vh(kcd + 1),
                                 at1[:, h * RB:h * RB + 384],
                                 start=False, stop=False, skip_group_check=True)
                nc.tensor.matmul(av[b0:b0 + 64, 256:RB], vh(kcd + 2),
                                 at2[:, h * RB:h * RB + 256],
                                 start=False, stop=False, skip_group_check=True)
                nc.tensor.matmul(av[b0:b0 + 64, 384:RB], vh(kcd + 3),
                                 at2[:, h * RB + 256:h * RB + 384],
                                 start=False, stop=True, skip_group_check=True)
            nc.vector.tensor_copy(ao_sb[:, p, q0:q0 + RB], av[:])

        # ------------- epilogue ---------------------------------------
        # ONE AllGather per query block carries [ao_own | silu(U)_own];
        # stats/LN/gate run fully locally after the gather (no second
        # collective, no cross-core stats dependency).
        agouts = {}
        loaded = {}

        def epilogue_a(key, q0, nq):
            """blocks 0-1: gather [ao | silu(U)] own halves."""
            sl = slice(q0, q0 + nq)
            agin = dram.tile([6, 128, nq], BF16, tag=f"agin{nq}")
            agout = dram.tile([2, 6, 128, nq], BF16, tag=f"agout{nq}")
            nc.gpsimd.dma_start(out=agin[0:3].rearrange("p i j -> i p j"),
                                in_=ao_sb[:, :, sl])
            nc.gpsimd.dma_start(out=agin[3:6].rearrange("p i j -> i p j"),
                                in_=ut_sb[:, :, sl])
            nc.gpsimd.collective_compute(
                "AllGather", mybir.AluOpType.bypass, replica_groups=pairs,
                ins=[agin.opt()], outs=[agout.opt()])
            agouts[key] = agout

        def epilogue_a2():
            """block 2: silu(U) is local (ut2), gather ao only."""
            sl = slice(2 * RB, 3 * RB)
            agin = dram.tile([3, 128, RB], BF16, tag="agin2")
            agout = dram.tile([2, 3, 128, RB], BF16, tag="agout2")
            nc.gpsimd.dma_start(out=agin.rearrange("p i j -> i p j"),
                                in_=ao_sb[:, :, sl])
            nc.gpsimd.collective_compute(
                "AllGather", mybir.AluOpType.bypass, replica_groups=pairs,
                ins=[agin.opt()], outs=[agout.opt()])
            agouts[2] = agout

        def epilogue_a3(p):
            """per-pair qb3 ao-only gather: fires as pair p's ao lands."""
            sl = slice(3 * RB, 4 * RB)
            agin = dram.tile([128, RB], BF16, tag="agin3", bufs=3)
            agout = dram.tile([2, 128, RB], BF16, tag="agout3", bufs=3)
            nc.gpsimd.dma_start(out=agin[:], in_=ao_sb[:, p, sl])
            nc.gpsimd.collective_compute(
                "AllGather", mybir.AluOpType.bypass, replica_groups=pairs,
                ins=[agin.opt()], outs=[agout.opt()])
            agouts[(3, p)] = agout

        def epilogue_b1(key, qb, sb3, agoff=0):
            """prefetch: residual + gathered ao (and ut for blocks 0-1)."""
            rt3 = sb3.tile([128, 3, RB], BF16, tag="rt3", bufs=2)
            nc.scalar.dma_start(out=rt3[:],
                                in_=residT[:, :, qb * RB:(qb + 1) * RB])
            aof = sb3.tile([128, 2, 3, RB], BF16, tag="aof", bufs=2)
            agout = agouts[key]
            sl = slice(agoff, agoff + RB)
            if key == 2:
                for r in range(2):
                    nc.sync.dma_start(
                        out=aof[:, r, :, :],
                        in_=agout[r].rearrange("p i j -> i p j"))
                utfull = ut2_sb[:, :, 0:RB]
            else:
                utf = sb3.tile([128, 2, 3, RB], BF16, tag="utf", bufs=1)
                for r in range(2):
                    nc.sync.dma_start(
                        out=aof[:, r, :, :],
                        in_=agout[r, 0:3, :, sl].rearrange("p i j -> i p j"))
                    nc.sync.dma_start(
                        out=utf[:, r, :, :],
                        in_=agout[r, 3:6, :, sl].rearrange("p i j -> i p j"))
                utfull = utf.rearrange("i r p j -> i (r p) j")
            loaded[(key, qb)] = (aof, utfull, rt3)

        def finish_ln(qb, st, aofull, utfull, rt3, sb3, ssb):
            """stats rows of `st` -> LN -> gate -> out-proj -> store.

            The per-token rstd commutes out of the hidden contraction, so
            gated = (ao - mu) * ut (2 ops/ct) and rstd multiplies the
            projected PSUM at the end; mu/rstd reach all 128 partitions
            via a ones[1,128] PE matmul into the same `st` bank (no
            GpSimd broadcast in the latency chain).
            """
            mvm = ssb.tile([1, RB], F32, tag="mvm")
            mu_b = ssb.tile([1, RB], BF16, tag="mub")
            nc.vector.tensor_scalar_mul(mvm[:], st[0:1, :], 1.0 / HID)
            nc.vector.tensor_copy(mu_b[:], mvm[:])
            mvq = ssb.tile([1, RB], F32, tag="mvq")
            musq = ssb.tile([1, RB], F32, tag="musq")
            nc.vector.tensor_scalar_mul(mvq[:], st[32:33, :], 1.0 / HID)
            nc.vector.tensor_mul(musq[:], mvm[:], mvm[:])
            nc.vector.tensor_sub(mvq[:], mvq[:], musq[:])
            std = ssb.tile([1, RB], F32, tag="std")
            rstd = ssb.tile([1, RB], F32, tag="rstd")
            rstd_b = ssb.tile([1, RB], BF16, tag="rstdb")
            nc.scalar.activation(std[:], mvq[:], AF.Sqrt, bias=eps_t[:])
            nc.vector.reciprocal_approx_fast(rstd[:], std[:])
            nc.vector.tensor_copy(rstd_b[:], rstd[:])
            # broadcast mu into st (PE), gate, then broadcast rstd
            nc.tensor.matmul(st[:], ones_r_sb[:], mu_b[:],
                             start=True, stop=True, skip_group_check=True)
            gated = sb3.tile([128, 6, RB], BF16, tag="gated")
            for ct in range(6):
                # subs read the PSUM broadcast (DVE only); the SBUF-only
                # muls alternate DVE / Pool so the chain paces at ~1.5
                # ops per ct instead of 2
                d1 = sb3.tile([128, RB], BF16, tag="d1", name=f"d1{ct % 2}")
                nc.vector.tensor_sub(d1[:], aofull[:, ct, :], st[:])
                eng = nc.vector if ct % 2 == 0 else nc.gpsimd
                eng.tensor_mul(gated[:, ct, :], d1[:], utfull[:, ct, :])
            nc.tensor.matmul(st[:], ones_r_sb[:], rstd_b[:],
                             start=True, stop=True, skip_group_check=True)
            rs_s = sb3.tile([128, RB], BF16, tag="rss")
            nc.vector.tensor_copy(rs_s[:], st[:])
            o_all = sb3.tile([128, 3, RB], F32, tag="oall")
            for ctp in range(3):
                po = pp.tile([128, RB], F32, tag="pp", name="po")
                for ct in range(6):
                    nc.tensor.matmul(
                        po[:], wout_sb[:, ct, ctp * 128:(ctp + 1) * 128],
                        gated[:, ct, :], start=(ct == 0), stop=(ct == 5))
                d3 = sb3.tile([128, RB], F32, tag="d3")
                nc.vector.tensor_mul(d3[:], po[:], rs_s[:])
                eng = nc.vector if ctp % 2 == 0 else nc.gpsimd
                eng.tensor_add(o_all[:, ctp, :], d3[:], rt3[:, ctp, :])
            nc.sync.dma_start(out=out[:, qb], in_=o_all[:])

        def epilogue_b(key, qb, sb3, ssb):
            aof, utfull, rt3 = loaded[(key, qb)]
            aofull = aof.rearrange("i r p j -> i (r p) j")    # [128, 6, RB]
            st = opo.tile([128, RB], F32, tag="st")
            for ct in range(6):
                nc.tensor.matmul(st[0:1, :], ones_k_sb[:], aofull[:, ct, :],
                                 start=(ct == 0), stop=(ct == 5),
                                 skip_group_check=True)
            sq = sb3.tile([128, 6, RB], BF16, tag="sq")
            for ct in range(6):
                nc.vector.tensor_mul(sq[:, ct, :], aofull[:, ct, :],
                                     aofull[:, ct, :])
            for ct in range(6):
                nc.tensor.matmul(st[32:33, :], ones_k_sb[:], sq[:, ct, :],
                                 start=(ct == 0), stop=(ct == 5),
                                 skip_group_check=True)
            finish_ln(qb, st, aofull, utfull, rt3, sb3, ssb)

        # --- block 3: per-pair loads + stats as each pair's gather lands
        b3 = {}

        def epilogue_b3_pre(sb3):
            b3["st"] = opo.tile([128, RB], F32, tag="st", name="st3")
            b3["aof"] = sb3.tile([128, 2, 3, RB], BF16, tag="aof", bufs=2,
                                 name="aof3")
            b3["rt3"] = sb3.tile([128, 3, RB], BF16, tag="rt3", bufs=2,
                                 name="rt33")
            nc.scalar.dma_start(out=b3["rt3"][:],
                                in_=residT[:, :, 3 * RB:4 * RB])

        def epilogue_b3_pair(p, sb3):
            agout = agouts[(3, p)]
            aof, st = b3["aof"], b3["st"]
            for r in range(2):
                nc.sync.dma_start(out=aof[:, r, p, :], in_=agout[r])
            for r in range(2):
                nc.tensor.matmul(st[0:1, :], ones_k_sb[:], aof[:, r, p, :],
                                 start=(p == 0 and r == 0),
                                 stop=(p == 2 and r == 1),
                                 skip_group_check=True)
            sq = sb3.tile([128, 2, RB], BF16, tag="sq3", bufs=2)
            for r in range(2):
                nc.vector.tensor_mul(sq[:, r, :], aof[:, r, p, :],
                                     aof[:, r, p, :])
            for r in range(2):
                nc.tensor.matmul(st[32:33, :], ones_k_sb[:], sq[:, r, :],
                                 start=(p == 0 and r == 0),
                                 stop=(p == 2 and r == 1),
                                 skip_group_check=True)

        def epilogue_b3_rest(sb3, ssb):
            aofull = b3["aof"].rearrange("i r p j -> i (r p) j")
            finish_ln(3, b3["st"], aofull, ut2_sb[:, :, RB:2 * RB],
                      b3["rt3"], sb3, ssb)

        # ------------- emission ---------------------------------------
        # ------------- emission ---------------------------------------
        # epilogue A (gather trigger) fires one attention block after its
        # data is ready; epilogue B1/B one block later still, so
        # collective latency hides under the next block's attention.
        opo = es.enter_context(tc.tile_pool(name="opo", bufs=1, space="PSUM"))
        sb3 = es.enter_context(tc.tile_pool(name="p3sb", bufs=1))
        ssb = es.enter_context(tc.tile_pool(name="p3small", bufs=1))

        def interleave(qb, units, extra=()):
            """attention pairs of qb round-robined with proj/epilogue
            units so PE slack inside the ACT-paced attention is filled."""
            units = list(units) + list(extra)
            n = len(units)
            cuts = [n // 3 + (1 if i < n % 3 else 0) for i in range(3)]
            i = 0
            for p in range(NPAIR):
                attn_pair(qb, p)
                for _ in range(cuts[p]):
                    units[i]()
                    i += 1

        proj_block(pp, 0)
        interleave(0, proj_units(pp, 1))
        interleave(1, proj_units(pp, 2),
                   [lambda: epilogue_a("01", 0, 2 * RB)])
        interleave(2, proj_units(pp, 3),
                   [lambda: epilogue_a2(),
                    lambda: epilogue_b1("01", 0, sb3, agoff=0),
                    lambda: epilogue_b("01", 0, sb3, ssb),
                    lambda: epilogue_b1("01", 1, sb3, agoff=RB),
                    lambda: epilogue_b("01", 1, sb3, ssb)])
        epilogue_b1(2, 2, sb3)
        attn_pair(3, 0)
        epilogue_a3(0)
        epilogue_b(2, 2, sb3, ssb)
        attn_pair(3, 1)
        epilogue_a3(1)
        epilogue_b3_pre(sb3)
        attn_pair(3, 2)
        epilogue_a3(2)
        epilogue_b3_pair(0, sb3)
        epilogue_b3_pair(1, sb3)
        # keep the PE HAM-warm through the exposed pair-2 gather window
        # so the final stats/out-proj matmuls run at 2.4 GHz, not 1.2
        wu2t = avp.tile([128, RB], F32, tag="av", name="warmtail")
        for i in range(40):
            nc.tensor.matmul(wu2t[:], warm_sb[:, 0:128], warm_sb[:],
                             start=(i == 0), stop=(i == 39))
        epilogue_b3_pair(2, sb3)
        epilogue_b3_rest(sb3, ssb)


# ---------------------------------------------------------------------------
# host side
# ---------------------------------------------------------------------------

def prep_inputs(x, attn_mask, W_proj, b_proj, ln_gamma, ln_beta, W_out, b_out):
    x = np.asarray(x, dtype=np.float32)
    W_proj = np.asarray(W_proj, dtype=np.float32)
    b_proj = np.asarray(b_proj, dtype=np.float32)
    ln_gamma = np.asarray(ln_gamma, dtype=np.float32)
    ln_beta = np.asarray(ln_beta, dtype=np.float32)
    W_out = np.asarray(W_out, dtype=np.float32)
    b_out = np.asarray(b_out, dtype=np.float32)

    tril = np.tril(np.ones((S, S), dtype=bool))
    am = np.asarray(attn_mask)
    if not all(np.array_equal(am[b], tril) for b in range(am.shape[0])):
        raise ValueError("kernel specialized for causal attn_mask")
    if np.any(b_proj != 0) or np.any(ln_beta != 0):
        raise ValueError("kernel specialized for zero b_proj / ln_beta")

    bf = ml_dtypes.bfloat16
    cos, sin = _rope_tables()                          # [S, 64]
    cosT = np.ascontiguousarray(cos.T)                 # [64, S]
    # source-side rotate factor g: row d carries the factor applied to
    # Q[d] BEFORE the 32-block shift: +sin for d<32, -sin for d>=32
    sinfT = np.ascontiguousarray(sin.T).copy()
    sinfT[32:64] *= -1.0
    cosT2 = np.vstack([cosT, cosT]).astype(bf)         # [128, S]
    sinfT2 = np.vstack([sinfT, sinfT]).astype(bf)

    ii = np.arange(128)[:, None]
    maskf = (np.arange(128)[None, :] >= ii).astype(np.float32).astype(bf)
    ones_k = np.ones((128, 1), dtype=bf)

    Wg = (ln_gamma[:, None] * W_out).astype(np.float32)
    U_c, V_c, Q_c, K_c = 0, HID, 2 * HID, 3 * HID

    in_maps = []
    for c in range(N_CORES):
        b, hh = c // 2, c % 2
        heads = range(NH * hh, NH * hh + NH)
        qcols = np.concatenate(
            [np.arange(Q_c + h * D, Q_c + (h + 1) * D) for h in heads])
        kcols = qcols - Q_c + K_c
        vcols = qcols - Q_c + V_c
        ucols = np.arange(U_c + hh * 384, U_c + (hh + 1) * 384)
        w_qku = np.concatenate(
            [W_proj[:, qcols], W_proj[:, kcols], W_proj[:, ucols]],
            axis=1)                                    # [768, 1152]
        w_qku_pack = np.ascontiguousarray(
            w_qku.reshape(6, 128, 1152).transpose(1, 0, 2)).astype(bf)
        # all 768 U columns in global order (for the local-U blocks 2-3)
        w_u2_pack = np.ascontiguousarray(
            W_proj[:, U_c:U_c + HID]
            .reshape(6, 128, HID).transpose(1, 0, 2)).astype(bf)
        wv = W_proj[:, vcols]                          # [768, 384]
        wv_pack = np.ascontiguousarray(
            wv.reshape(6, 128, 384).transpose(1, 0, 2)).astype(bf)
        # own 384 gamma-folded out columns, packed [128, 6, 384]
        wout_pack = np.ascontiguousarray(
            Wg[:, hh * 384:(hh + 1) * 384]
            .reshape(6, 128, 384).transpose(1, 0, 2)).astype(bf)
        xTb = x[b].T                                   # [768, 2048]
        # packed [128, 4, 6, 512]: xp[p, nb, k, s] = xTb[k*128+p, nb*512+s]
        xp = np.ascontiguousarray(
            xTb.reshape(6, 128, 4, RB).transpose(1, 2, 0, 3)).astype(bf)
        # residual + b_out for own 384 out rows, packed [128, 3, 2048]
        resid = (xTb[hh * 384:(hh + 1) * 384, :]
                 + b_out[hh * 384:(hh + 1) * 384, None])   # [384, 2048]
        resid_pack = np.ascontiguousarray(
            resid.reshape(3, 128, S).transpose(1, 0, 2)).astype(bf)
        in_maps.append(dict(
            xp=xp,
            w_qku=w_qku_pack,
            w_u2=w_u2_pack,
            wv=wv_pack,
            w_out=wout_pack,
            cosT2=cosT2, sinfT2=sinfT2, maskf=maskf,
            ones_k=ones_k,
            residT=resid_pack,
        ))
    return in_maps


def assemble(results, B=4):
    full = np.empty((B, S, HID), dtype=np.float32)
    for c in range(N_CORES):
        b, hh = c // 2, c % 2
        o = results[c]["out"].reshape(128, 4, 3, RB)
        # out[p, qb, ctp, s] = y[qb*512 + s, hh*384 + ctp*128 + p]
        full[b, :, hh * 384:(hh + 1) * 384] = (
            o.transpose(1, 3, 2, 0).reshape(S, 384))
    return full


_NC_CACHE = {}


def get_nc(ndev=N_CORES):
    if ndev not in _NC_CACHE:
        pairs = [[i, i + 1] for i in range(0, ndev, 2)]
        _NC_CACHE[ndev] = build_nc(ndev, pairs)
    return _NC_CACHE[ndev]


def kernel(**inputs):
    in_maps = prep_inputs(**inputs)
    nc = get_nc(N_CORES)
    res = bass_utils.run_bass_kernel_spmd(
        nc, in_maps, core_ids=list(range(N_CORES)))
    return assemble(res.results)
